# revision 37
# baseline (speedup 1.0000x reference)
"""GAT layer (dense-adj variant) on 8 Trainium2 NeuronCores.

Row-parallel over destination nodes (R=1024 rows/core). Key structure vs the
old kernel: the score matrix E = exp(leaky_relu(src_i + dst_j)) * adj is
accumulated as G = E - 1 (G-decomposition), with the all-ones part folded in
exactly via a host-computed column sum of h:

  exp(leaky(z)) = max(exp(z), exp(0.01 z));  exp(z) = exp(src)*exp(dst)
  E - 1 = Prelu(exp(src_i)*exp(dst_j) - 1, alpha) * adj   (alpha~0.016 approx
          of the negative branch; end-to-end rel err ~1.8e-3)
  out_num[i,:] = hsum + sum_j G[j,i] h_aug[j,:]   (hsum = sum_j h_aug[j] from
          host x.sum(0) @ rhs, entering as two exact bf16 hi/lo K=1 matmuls
          that start each PSUM accumulation chain)
  out = out_num[:, :256]/Z + fc_b  (fc_b passes through softmax exactly since
          attention rows sum to 1); Z = out_num[:, 256].

Per j-strip cost: 1 ACT pass (Prelu, scale=exp(dst_j), bias=-1) + 1 mul by
adj (DVE or GPSIMD; GPSIMD-mul strips load adj as fp8 - GPSIMD rate is
dtype-blind, halving those strips' DMA). No per-element Exp/Prelu over the
full matrix, no softmax pass: ScalarE drops from ~105us to ~73us, DVE from
~110us to ~80us, adj DMA from 16.8MB to 13.3MB/core. PE is the wall
(~103us busy: C 57.5 + B 42 + LDW edges); B stays replicated because the
h all-gather alternatives measured too slow (CC AllGather ~20us/MB serial).

Phase B k-chains rotate over 4 full-bank-padded PSUM tiles (read-modify-
write on the same bank with rotation distance <=2 runs 40-60% slower), and
group g's elementwise is emitted while group g+1's B matmuls run. Pool
sizes (gstream=26, work=4) are load-bearing: larger pools increase engine
overlap and SBUF port contention (DVE tensor_tensor measured 620ns idle-PE
vs 1100-1700ns under load), measurably hurting wall time.

Measured: ~152us median (149.5-158 across runs, +-3%), rel err 1.67e-3
(vs 179.6us / 3.58e-3 for the previous kernel).
"""

import numpy as np
import ml_dtypes

N = 8192
IN_DIM = 512
OUT_DIM = 256
NCORES = 8
R = N // NCORES  # 1024 rows per core
KT = IN_DIM // 128  # 4 k-tiles
JT = N // 128  # 64 j-strips
IT = R // 128  # 8 i-tiles per core
HS = OUT_DIM + 2  # h_sb slot width  (h | ones | dst)
HA = OUT_DIM + 1  # C-matmul rhs width (h | ones)
GC = 8  # strips per dst-extraction chunk

ALPHA = 0.016  # Prelu negative-branch slope approximating exp(0.01 z)-1
N_GP = 26  # strips whose adj-mul runs on GPSIMD (their adj comes in fp8)

bf16 = ml_dtypes.bfloat16
f8 = ml_dtypes.float8_e4m3fn

_cache = {}


def _gp_strips():
    return set(np.linspace(0, JT - 1, N_GP).astype(int).tolist()) if N_GP else set()


def _build():
    import concourse.tile as tile
    from concourse import bacc, mybir

    AF = mybir.ActivationFunctionType
    ALU = mybir.AluOpType
    f32 = mybir.dt.float32
    bft = mybir.dt.bfloat16
    f8t = mybir.dt.float8e4

    gp_strips = _gp_strips()

    nc = bacc.Bacc("TRN2", num_devices=NCORES, target_bir_lowering=False, debug=False)

    # adj strips: bf16 for DVE-mul strips (a 1-byte operand forces the DVE out
    # of 2x mode, ~1.7x slower in situ), fp8 for GPSIMD-mul strips (dtype-
    # blind engine, so the halved DMA is free there).
    n_gp = len(gp_strips)
    adjTb_d = nc.dram_tensor("adjTb", [(JT - n_gp) * 128, R], bft, kind="ExternalInput").ap()
    adjT8_d = (
        nc.dram_tensor("adjT8", [n_gp * 128, R], f8t, kind="ExternalInput").ap()
        if n_gp
        else None
    )
    xT_d = nc.dram_tensor("xT", [IN_DIM, N], bft, kind="ExternalInput").ap()
    xTi_d = nc.dram_tensor("xTi", [IN_DIM, R], bft, kind="ExternalInput").ap()
    # rhs_aug columns: [fc_w (256) | zeros (1) | a_dst (1)]
    rhs_aug_d = nc.dram_tensor("rhs_aug", [IN_DIM, HS], bft, kind="ExternalInput").ap()
    w_src_rep_d = nc.dram_tensor("w_src_rep", [IN_DIM, 128], bft, kind="ExternalInput").ap()
    src_bias_d = nc.dram_tensor("src_bias", [128, 1], f32, kind="ExternalInput").ap()
    # hsum replicated (hi in rows 0-63, lo in rows 64-127): one K=128 matmul
    # against a constant (1/64) lhsT adds hi+lo exactly into each acc chain.
    hsum_d = nc.dram_tensor("hsum", [128, HA], bft, kind="ExternalInput").ap()
    fcb_rep_d = nc.dram_tensor("fcb_rep", [128, OUT_DIM], f32, kind="ExternalInput").ap()
    out_d = nc.dram_tensor("out", [R, OUT_DIM], f32, kind="ExternalOutput").ap()

    with tile.TileContext(nc) as tc:
        with (
            tc.tile_pool(name="const", bufs=1) as cpool,
            tc.tile_pool(name="hpool", bufs=1) as hpool,
            tc.tile_pool(name="xstream", bufs=3) as xpool,
            tc.tile_pool(name="abstream", bufs=10) as abpool,
            tc.tile_pool(name="a8stream", bufs=6) as a8pool,
            tc.tile_pool(name="gstream", bufs=34) as gpool,
            tc.tile_pool(name="work", bufs=4) as wpool,
            tc.tile_pool(name="opool", bufs=3) as opool,
        ):
            # ---- constants (xTi/w_src first: phase A gates the first Prelu) ----
            src_bias_sb = cpool.tile([128, 1], f32)
            nc.sync.dma_start(src_bias_sb[:], src_bias_d)
            w_src_sb = cpool.tile([128, KT * 128], bft)
            nc.sync.dma_start(
                w_src_sb[:].rearrange("p (k n) -> p k n", k=KT),
                w_src_rep_d.rearrange("(k p) n -> p k n", p=128),
            )
            xTi_sb = cpool.tile([128, KT * R], bft)
            for half in range(2):
                nc.sync.dma_start(
                    xTi_sb[:].rearrange("p (k n) -> p k n", k=KT)[
                        :, :, half * 512 : (half + 1) * 512
                    ],
                    xTi_d.rearrange("(k p) n -> p k n", p=128)[
                        :, :, half * 512 : (half + 1) * 512
                    ],
                )
            rhs_aug_sb = cpool.tile([128, KT * HS], bft)
            nc.sync.dma_start(
                rhs_aug_sb[:].rearrange("p (k n) -> p k n", k=KT),
                rhs_aug_d.rearrange("(k p) n -> p k n", p=128),
            )
            hsum_sb = cpool.tile([128, HA], bft)
            nc.sync.dma_start(hsum_sb[:], hsum_d)
            fcb_rep_sb = cpool.tile([128, OUT_DIM], f32)
            nc.sync.dma_start(fcb_rep_sb[:], fcb_rep_d)
            inv64_sb = cpool.tile([128, 128], bft)
            nc.vector.memset(inv64_sb[:], 1.0 / 64.0)
            neg1_sb = cpool.tile([128, 1], f32)
            nc.vector.memset(neg1_sb[:], -1.0)

            src_rep = cpool.tile([128, R], bft)
            esrc_rep = cpool.tile([128, R], bft)
            h_sb = hpool.tile([128, JT * HS], bft)
            dst_sb = cpool.tile([128, JT], f32)
            edst_sb = cpool.tile([128, JT], f32)
            g_strips = [None] * JT

            ps_ab_cm = tc.tile_pool(name="ps_ab", bufs=8, space="PSUM")
            ps_ab = ps_ab_cm.__enter__()

            # ---- Phase B (replicated h) + per-strip elementwise ----
            bi = 0  # running index into adjTb
            g8i = 0  # running index into adjT8
            adj_tiles = [None] * JT

            def c_adj_dma(jt):
                nonlocal bi, g8i
                if jt in gp_strips:
                    adjt = a8pool.tile([128, R], f8t, name="adj8")
                    nc.sync.dma_start(adjt[:], adjT8_d[g8i * 128 : (g8i + 1) * 128, :])
                    g8i += 1
                else:
                    adjt = abpool.tile([128, R], bft, name="adjb")
                    nc.sync.dma_start(adjt[:], adjTb_d[bi * 128 : (bi + 1) * 128, :])
                    bi += 1
                adj_tiles[jt] = adjt

            def c_elementwise(jt):
                # G[j,i] strip: Prelu(exp(src)*exp(dst_j) - 1, alpha) * adj
                pre = wpool.tile([128, R], bft, name="pre", tag="pre")
                nc.scalar.activation(
                    pre[:], esrc_rep[:], AF.Prelu,
                    bias=neg1_sb[:], scale=edst_sb[:, jt : jt + 1], alpha=ALPHA,
                )
                g = gpool.tile([128, R], bft, name="g")
                if jt in gp_strips:
                    nc.gpsimd.tensor_mul(g[:], pre[:], adj_tiles[jt][:])
                else:
                    nc.vector.tensor_mul(g[:], pre[:], adj_tiles[jt][:])
                g_strips[jt] = g

            def c_group_post(g0):
                # ones column + dst extraction + exp for group g0's slots
                nc.vector.memset(
                    h_sb[:, g0 * GC * HS : (g0 + 1) * GC * HS].rearrange(
                        "p (j s) -> p j s", s=HS
                    )[:, :, OUT_DIM : OUT_DIM + 1],
                    1.0,
                )
                nc.vector.tensor_copy(
                    dst_sb[:, g0 * GC : (g0 + 1) * GC],
                    h_sb[:, g0 * GC * HS : (g0 + 1) * GC * HS].rearrange(
                        "p (j s) -> p j s", s=HS
                    )[:, :, HS - 1 : HS],
                )
                nc.scalar.activation(
                    edst_sb[:, g0 * GC : (g0 + 1) * GC],
                    dst_sb[:, g0 * GC : (g0 + 1) * GC],
                    AF.Exp,
                )

            # 8 strips (one group) per iteration; ps tiles padded to [128,512]
            # f32 = one full 2KB bank each, so the k-chains rotate over all 8
            # PSUM banks.
            def b_group(jq):
                xTj = xpool.tile([128, KT * GC * 128], bft)
                nc.sync.dma_start(
                    xTj[:].rearrange("p (k n) -> p k n", k=KT),
                    xT_d[:, jq * GC * 128 : (jq + 1) * GC * 128].rearrange(
                        "(k p) n -> p k n", p=128
                    ),
                )
                ps8 = [ps_ab.tile([128, 512], f32, name=f"ps_b{u}", tag="ps")
                       for u in range(GC)]
                for kt in range(KT):
                    for u in range(GC):
                        nc.tensor.matmul(
                            ps8[u][:, 0:HS],
                            xTj[:, (kt * GC + u) * 128 : (kt * GC + u + 1) * 128],
                            rhs_aug_sb[:, kt * HS : (kt + 1) * HS],
                            start=(kt == 0),
                            stop=(kt == KT - 1),
                        )
                # slot: [h (256) | 1.0 | dst]; rhs_aug col 256 is 0 so the copy
                # writes 0 there; a strided memset per group sets the ones.
                for u in range(GC):
                    jt_ = jq * GC + u
                    nc.vector.tensor_copy(
                        h_sb[:, jt_ * HS : (jt_ + 1) * HS], ps8[u][:, 0:HS]
                    )
                    c_adj_dma(jt_)
                c_group_post(jq)

            # Group 0's B matmuls go first in the PE stream: phase A gates only
            # the Prelus (ACT), so starting B before A removes the xTi-DMA ->
            # A -> B serialization from the in-order PE at startup.
            b_group(0)

            # ---- Phase A: src_rep[p, f] = src[i0+f] for all p; esrc = exp ----
            ps_a = [ps_ab.tile([128, 512], f32, name=f"ps_a{ch}", tag="ps")
                    for ch in range(R // 512)]
            for kt in range(KT):
                for ch in range(R // 512):
                    nc.tensor.matmul(
                        ps_a[ch][:],
                        w_src_sb[:, kt * 128 : (kt + 1) * 128],
                        xTi_sb[:, kt * R + ch * 512 : kt * R + (ch + 1) * 512],
                        start=(kt == 0),
                        stop=(kt == KT - 1),
                    )
            for ch in range(R // 512):
                nc.scalar.activation(
                    src_rep[:, ch * 512 : (ch + 1) * 512], ps_a[ch][:], AF.Identity,
                    bias=src_bias_sb[:],
                )
            nc.scalar.activation(esrc_rep[:], src_rep[:], AF.Exp)

            for s_jt in range(GC):
                c_elementwise(s_jt)
            for jq in range(1, JT // GC):
                b_group(jq)
                for s_jt in range(jq * GC, (jq + 1) * GC):
                    c_elementwise(s_jt)

            # ---- Phase C: 8 PSUM bank accumulators, hsum-start + 64 strips ----
            ps_ab_cm.__exit__(None, None, None)
            out_ps = {}
            with tc.tile_pool(name="ps_acc", bufs=1, space="PSUM") as ps_acc:
                for it in range(IT):
                    out_ps[it] = ps_acc.tile(
                        [128, HA], f32, name=f"acc{it}", tag=f"acc{it}"
                    )
                for it in range(IT):
                    # acc := (1/64) * ones^T @ hsum_rep = hsum_hi + hsum_lo
                    nc.tensor.matmul(
                        out_ps[it][:], inv64_sb[:], hsum_sb[:],
                        start=True, stop=False,
                    )
                for jt in range(JT):
                    g = g_strips[jt]
                    hj = h_sb[:, jt * HS : jt * HS + HA]
                    for it in range(IT):
                        nc.tensor.matmul(
                            out_ps[it][:],
                            g[:, it * 128 : (it + 1) * 128],
                            hj,
                            start=False,
                            stop=(jt == JT - 1),
                        )
                        if jt == JT - 1:
                            # Phase D interleaved: acc `it` is final here, so
                            # out = num/Z + fc_b starts while accs it+1..7 are
                            # still accumulating their last strip.
                            rz = opool.tile([128, 1], f32, tag="rz")
                            nc.vector.reciprocal(
                                rz[:], out_ps[it][:, OUT_DIM : OUT_DIM + 1]
                            )
                            o = opool.tile([128, OUT_DIM], f32, tag="o")
                            nc.vector.tensor_scalar_mul(
                                o[:], out_ps[it][:, 0:OUT_DIM], rz[:]
                            )
                            o2 = opool.tile([128, OUT_DIM], f32, tag="o2")
                            nc.vector.tensor_add(o2[:], o[:], fcb_rep_sb[:])
                            nc.sync.dma_start(
                                out_d[it * 128 : (it + 1) * 128, :], o2[:]
                            )

    nc.compile()
    return nc


def _prep_inputs(adj, x, fc_w, fc_b, attn_w, attn_b):
    fc_w = np.asarray(fc_w, np.float32)
    fc_b = np.asarray(fc_b, np.float32)
    attn_w = np.asarray(attn_w, np.float32)
    x = np.asarray(x, np.float32)
    a_src = fc_w @ attn_w[:OUT_DIM]
    a_dst = fc_w @ attn_w[OUT_DIM:]
    b_src = float(fc_b @ attn_w[:OUT_DIM]) + float(attn_b)
    b_dst = float(fc_b @ attn_w[OUT_DIM:])

    xT = np.ascontiguousarray(x.T).astype(bf16)
    adjT = np.asarray(adj, np.float32).T  # [N (src j), N (dest i)]
    rhs_aug = np.concatenate(
        [fc_w, np.zeros((IN_DIM, 1), np.float32), a_dst[:, None]], axis=1
    ).astype(bf16)
    w_src_rep = np.tile(a_src[:, None], (1, 128)).astype(bf16)
    src_bias = np.full((128, 1), b_src, np.float32)

    # hsum = sum_j h_aug[j] over the de-biased h0 = x@fc_w (fc_b added in D);
    # dst column of h_aug includes b_dst, ones column sums to N.
    xsum = x.sum(0).astype(np.float64)
    hsum_h = xsum @ fc_w.astype(np.float64)  # [256]
    hsum_full = np.concatenate([hsum_h, [float(N)]]).astype(np.float32)  # [257]
    hsum_hi = hsum_full.astype(bf16)
    hsum_lo = (hsum_full - hsum_hi.astype(np.float32)).astype(bf16)
    # rows 0-63 = hi, rows 64-127 = lo; contracted against a (1/64) lhsT
    hsum = np.concatenate(
        [np.tile(hsum_hi[None, :], (64, 1)), np.tile(hsum_lo[None, :], (64, 1))]
    ).astype(bf16)  # [128, 257]
    fcb_rep = np.tile(fc_b[None, :], (128, 1)).astype(np.float32)

    gp_strips = sorted(_gp_strips())
    bf_strips = [j for j in range(JT) if j not in set(gp_strips)]
    in_maps = []
    for c in range(NCORES):
        sl = slice(c * R, (c + 1) * R)
        adjTc = adjT[:, sl]
        adjTb = np.concatenate(
            [adjTc[j * 128 : (j + 1) * 128] for j in bf_strips], axis=0
        ).astype(bf16)
        m = {
            "adjTb": np.ascontiguousarray(adjTb),
            "xT": xT,
            "xTi": np.ascontiguousarray(xT[:, sl]),
            "rhs_aug": rhs_aug,
            "w_src_rep": w_src_rep,
            "src_bias": src_bias,
            "hsum": hsum,
            "fcb_rep": fcb_rep,
        }
        if gp_strips:
            adjT8 = np.concatenate(
                [adjTc[j * 128 : (j + 1) * 128] for j in gp_strips], axis=0
            ).astype(f8)
            m["adjT8"] = np.ascontiguousarray(adjT8)
        in_maps.append(m)
    return in_maps


def kernel(adj, x, fc_w, fc_b, attn_w, attn_b, _trace=False, _tmpdir=None):
    from concourse import bass_utils

    if "nc" not in _cache:
        _cache["nc"] = _build()
    nc = _cache["nc"]
    in_maps = _prep_inputs(adj, x, fc_w, fc_b, attn_w, attn_b)
    res = bass_utils.run_bass_kernel_spmd(
        nc,
        in_maps,
        core_ids=list(range(NCORES)),
        trace=_trace,
        **({"tmpdir": _tmpdir} if _tmpdir else {}),
    )
    out = np.concatenate([res.results[c]["out"] for c in range(NCORES)], axis=0)
    if _trace:
        _cache["last_exec_time_ns"] = res.exec_time_ns
        _cache["last_profile_json"] = res.profile_json
    return out


# revision 39
# speedup vs baseline: 1.0473x; 1.0473x over previous
"""GAT layer (dense-adj variant) on 8 Trainium2 NeuronCores.

Row-parallel over destination nodes (R=1024 rows/core). Key structure vs the
old kernel: the score matrix E = exp(leaky_relu(src_i + dst_j)) * adj is
accumulated as G = E - 1 (G-decomposition), with the all-ones part folded in
exactly via a host-computed column sum of h:

  exp(leaky(z)) = max(exp(z), exp(0.01 z));  exp(z) = exp(src)*exp(dst)
  E - 1 = Prelu(exp(src_i)*exp(dst_j) - 1, alpha) * adj   (alpha~0.016 approx
          of the negative branch; end-to-end rel err ~1.8e-3)
  out_num[i,:] = hsum + sum_j G[j,i] h_aug[j,:]   (hsum = sum_j h_aug[j] from
          host x.sum(0) @ rhs, entering as two exact bf16 hi/lo K=1 matmuls
          that start each PSUM accumulation chain)
  out = out_num[:, :256]/Z + fc_b  (fc_b passes through softmax exactly since
          attention rows sum to 1); Z = out_num[:, 256].

Per j-strip cost: 1 ACT pass (Prelu, scale=exp(dst_j), bias=-1) + 1 mul by
adj (DVE or GPSIMD; GPSIMD-mul strips load adj as fp8 - GPSIMD rate is
dtype-blind, halving those strips' DMA). No per-element Exp/Prelu over the
full matrix, no softmax pass: ScalarE drops from ~105us to ~73us, DVE from
~110us to ~80us, adj DMA from 16.8MB to 13.3MB/core. PE is the wall
(~103us busy: C 57.5 + B 42 + LDW edges); B stays replicated because the
h all-gather alternatives measured too slow (CC AllGather ~20us/MB serial).

Phase B k-chains rotate over 4 full-bank-padded PSUM tiles (read-modify-
write on the same bank with rotation distance <=2 runs 40-60% slower), and
group g's elementwise is emitted while group g+1's B matmuls run. Pool
sizes (gstream=26, work=4) are load-bearing: larger pools increase engine
overlap and SBUF port contention (DVE tensor_tensor measured 620ns idle-PE
vs 1100-1700ns under load), measurably hurting wall time.

Measured: ~152us median (149.5-158 across runs, +-3%), rel err 1.67e-3
(vs 179.6us / 3.58e-3 for the previous kernel).
"""

import numpy as np
import ml_dtypes

N = 8192
IN_DIM = 512
OUT_DIM = 256
NCORES = 8
R = N // NCORES  # 1024 rows per core
KT = IN_DIM // 128  # 4 k-tiles
JT = N // 128  # 64 j-strips
IT = R // 128  # 8 i-tiles per core
HS = OUT_DIM + 2  # h_sb slot width  (h | ones | dst)
HA = OUT_DIM + 1  # C-matmul rhs width (h | ones)
GC = 8  # strips per dst-extraction chunk

ALPHA = 0.016  # Prelu negative-branch slope approximating exp(0.01 z)-1
N_GP = 26  # strips whose adj-mul runs on GPSIMD (their adj comes in fp8)

bf16 = ml_dtypes.bfloat16
f8 = ml_dtypes.float8_e4m3fn

_cache = {}


def _gp_strips():
    return set(np.linspace(0, JT - 1, N_GP).astype(int).tolist()) if N_GP else set()


def _build():
    import concourse.tile as tile
    from concourse import bacc, mybir

    AF = mybir.ActivationFunctionType
    ALU = mybir.AluOpType
    f32 = mybir.dt.float32
    bft = mybir.dt.bfloat16
    f8t = mybir.dt.float8e4

    gp_strips = _gp_strips()

    nc = bacc.Bacc("TRN2", num_devices=NCORES, target_bir_lowering=False, debug=False)

    # adj strips: bf16 for DVE-mul strips (a 1-byte operand forces the DVE out
    # of 2x mode, ~1.7x slower in situ), fp8 for GPSIMD-mul strips (dtype-
    # blind engine, so the halved DMA is free there).
    n_gp = len(gp_strips)
    adjTb_d = nc.dram_tensor("adjTb", [(JT - n_gp) * 128, R], bft, kind="ExternalInput").ap()
    adjT8_d = (
        nc.dram_tensor("adjT8", [n_gp * 128, R], f8t, kind="ExternalInput").ap()
        if n_gp
        else None
    )
    xT_d = nc.dram_tensor("xT", [IN_DIM, N], bft, kind="ExternalInput").ap()
    xTi_d = nc.dram_tensor("xTi", [IN_DIM, R], bft, kind="ExternalInput").ap()
    # rhs_aug columns: [fc_w (256) | zeros (1) | a_dst (1)]
    rhs_aug_d = nc.dram_tensor("rhs_aug", [IN_DIM, HS], bft, kind="ExternalInput").ap()
    w_src_rep_d = nc.dram_tensor("w_src_rep", [IN_DIM, 128], bft, kind="ExternalInput").ap()
    src_bias_d = nc.dram_tensor("src_bias", [128, 1], f32, kind="ExternalInput").ap()
    # hsum replicated (hi in rows 0-63, lo in rows 64-127): one K=128 matmul
    # against a constant (1/64) lhsT adds hi+lo exactly into each acc chain.
    hsum_d = nc.dram_tensor("hsum", [128, HA], bft, kind="ExternalInput").ap()
    fcb_rep_d = nc.dram_tensor("fcb_rep", [128, OUT_DIM], f32, kind="ExternalInput").ap()
    out_d = nc.dram_tensor("out", [R, OUT_DIM], f32, kind="ExternalOutput").ap()

    with tile.TileContext(nc) as tc:
        with (
            tc.tile_pool(name="const", bufs=1) as cpool,
            tc.tile_pool(name="hpool", bufs=1) as hpool,
            tc.tile_pool(name="xstream", bufs=3) as xpool,
            tc.tile_pool(name="abstream", bufs=10) as abpool,
            tc.tile_pool(name="a8stream", bufs=6) as a8pool,
            tc.tile_pool(name="gstream", bufs=26) as gpool,
            tc.tile_pool(name="work", bufs=4) as wpool,
            tc.tile_pool(name="opool", bufs=3) as opool,
        ):
            # ---- constants (xTi/w_src first: phase A gates the first Prelu) ----
            src_bias_sb = cpool.tile([128, 1], f32)
            nc.sync.dma_start(src_bias_sb[:], src_bias_d)
            w_src_sb = cpool.tile([128, KT * 128], bft)
            nc.sync.dma_start(
                w_src_sb[:].rearrange("p (k n) -> p k n", k=KT),
                w_src_rep_d.rearrange("(k p) n -> p k n", p=128),
            )
            xTi_sb = cpool.tile([128, KT * R], bft)
            for half in range(2):
                nc.sync.dma_start(
                    xTi_sb[:].rearrange("p (k n) -> p k n", k=KT)[
                        :, :, half * 512 : (half + 1) * 512
                    ],
                    xTi_d.rearrange("(k p) n -> p k n", p=128)[
                        :, :, half * 512 : (half + 1) * 512
                    ],
                )
            rhs_aug_sb = cpool.tile([128, KT * HS], bft)
            nc.sync.dma_start(
                rhs_aug_sb[:].rearrange("p (k n) -> p k n", k=KT),
                rhs_aug_d.rearrange("(k p) n -> p k n", p=128),
            )
            hsum_sb = cpool.tile([128, HA], bft)
            nc.sync.dma_start(hsum_sb[:], hsum_d)
            fcb_rep_sb = cpool.tile([128, OUT_DIM], f32)
            nc.sync.dma_start(fcb_rep_sb[:], fcb_rep_d)
            inv64_sb = cpool.tile([128, 128], bft)
            nc.vector.memset(inv64_sb[:], 1.0 / 64.0)
            neg1_sb = cpool.tile([128, 1], f32)
            nc.vector.memset(neg1_sb[:], -1.0)

            src_rep = cpool.tile([128, R], bft)
            esrc_rep = cpool.tile([128, R], bft)
            h_sb = hpool.tile([128, JT * HS], bft)
            dst_sb = cpool.tile([128, JT], f32)
            edst_sb = cpool.tile([128, JT], f32)
            g_strips = [None] * JT

            ps_ab_cm = tc.tile_pool(name="ps_ab", bufs=8, space="PSUM")
            ps_ab = ps_ab_cm.__enter__()

            # ---- Phase B (replicated h) + per-strip elementwise ----
            bi = 0  # running index into adjTb
            g8i = 0  # running index into adjT8
            adj_tiles = [None] * JT

            def c_adj_dma(jt):
                nonlocal bi, g8i
                if jt in gp_strips:
                    adjt = a8pool.tile([128, R], f8t, name="adj8")
                    nc.sync.dma_start(adjt[:], adjT8_d[g8i * 128 : (g8i + 1) * 128, :])
                    g8i += 1
                else:
                    adjt = abpool.tile([128, R], bft, name="adjb")
                    nc.sync.dma_start(adjt[:], adjTb_d[bi * 128 : (bi + 1) * 128, :])
                    bi += 1
                adj_tiles[jt] = adjt

            def c_elementwise(jt):
                # G[j,i] strip: Prelu(exp(src)*exp(dst_j) - 1, alpha) * adj
                pre = wpool.tile([128, R], bft, name="pre", tag="pre")
                nc.scalar.activation(
                    pre[:], esrc_rep[:], AF.Prelu,
                    bias=neg1_sb[:], scale=edst_sb[:, jt : jt + 1], alpha=ALPHA,
                )
                g = gpool.tile([128, R], bft, name="g")
                if jt in gp_strips:
                    nc.gpsimd.tensor_mul(g[:], pre[:], adj_tiles[jt][:])
                else:
                    nc.vector.tensor_mul(g[:], pre[:], adj_tiles[jt][:])
                g_strips[jt] = g

            def c_group_post(g0):
                # ones column + dst extraction + exp for group g0's slots
                nc.vector.memset(
                    h_sb[:, g0 * GC * HS : (g0 + 1) * GC * HS].rearrange(
                        "p (j s) -> p j s", s=HS
                    )[:, :, OUT_DIM : OUT_DIM + 1],
                    1.0,
                )
                nc.vector.tensor_copy(
                    dst_sb[:, g0 * GC : (g0 + 1) * GC],
                    h_sb[:, g0 * GC * HS : (g0 + 1) * GC * HS].rearrange(
                        "p (j s) -> p j s", s=HS
                    )[:, :, HS - 1 : HS],
                )
                nc.scalar.activation(
                    edst_sb[:, g0 * GC : (g0 + 1) * GC],
                    dst_sb[:, g0 * GC : (g0 + 1) * GC],
                    AF.Exp,
                )

            # 8 strips (one group) per iteration; ps tiles padded to [128,512]
            # f32 = one full 2KB bank each, so the k-chains rotate over all 8
            # PSUM banks.
            def b_group(jq):
                xTj = xpool.tile([128, KT * GC * 128], bft)
                nc.sync.dma_start(
                    xTj[:].rearrange("p (k n) -> p k n", k=KT),
                    xT_d[:, jq * GC * 128 : (jq + 1) * GC * 128].rearrange(
                        "(k p) n -> p k n", p=128
                    ),
                )
                ps8 = [ps_ab.tile([128, 512], f32, name=f"ps_b{u}", tag="ps")
                       for u in range(GC)]
                for kt in range(KT):
                    for u in range(GC):
                        nc.tensor.matmul(
                            ps8[u][:, 0:HS],
                            xTj[:, (kt * GC + u) * 128 : (kt * GC + u + 1) * 128],
                            rhs_aug_sb[:, kt * HS : (kt + 1) * HS],
                            start=(kt == 0),
                            stop=(kt == KT - 1),
                        )
                # slot: [h (256) | 1.0 | dst]; rhs_aug col 256 is 0 so the copy
                # writes 0 there; a strided memset per group sets the ones.
                for u in range(GC):
                    jt_ = jq * GC + u
                    nc.vector.tensor_copy(
                        h_sb[:, jt_ * HS : (jt_ + 1) * HS], ps8[u][:, 0:HS]
                    )
                    c_adj_dma(jt_)
                c_group_post(jq)

            # ---- Phase A: src_rep[p, f] = src[i0+f] for all p; esrc = exp ----
            ps_a = [ps_ab.tile([128, 512], f32, name=f"ps_a{ch}", tag="ps")
                    for ch in range(R // 512)]
            for kt in range(KT):
                for ch in range(R // 512):
                    nc.tensor.matmul(
                        ps_a[ch][:],
                        w_src_sb[:, kt * 128 : (kt + 1) * 128],
                        xTi_sb[:, kt * R + ch * 512 : kt * R + (ch + 1) * 512],
                        start=(kt == 0),
                        stop=(kt == KT - 1),
                    )
            for ch in range(R // 512):
                nc.scalar.activation(
                    src_rep[:, ch * 512 : (ch + 1) * 512], ps_a[ch][:], AF.Identity,
                    bias=src_bias_sb[:],
                )
            nc.scalar.activation(esrc_rep[:], src_rep[:], AF.Exp)

            for jq in range(JT // GC):
                b_group(jq)
                for s_jt in range(jq * GC, (jq + 1) * GC):
                    c_elementwise(s_jt)

            # ---- Phase C: 8 PSUM bank accumulators, hsum-start + 64 strips ----
            ps_ab_cm.__exit__(None, None, None)
            out_ps = {}
            with tc.tile_pool(name="ps_acc", bufs=1, space="PSUM") as ps_acc:
                for it in range(IT):
                    out_ps[it] = ps_acc.tile(
                        [128, HA], f32, name=f"acc{it}", tag=f"acc{it}"
                    )
                for it in range(IT):
                    # acc := (1/64) * ones^T @ hsum_rep = hsum_hi + hsum_lo
                    nc.tensor.matmul(
                        out_ps[it][:], inv64_sb[:], hsum_sb[:],
                        start=True, stop=False,
                    )
                for jt in range(JT):
                    g = g_strips[jt]
                    hj = h_sb[:, jt * HS : jt * HS + HA]
                    for it in range(IT):
                        nc.tensor.matmul(
                            out_ps[it][:],
                            g[:, it * 128 : (it + 1) * 128],
                            hj,
                            start=False,
                            stop=(jt == JT - 1),
                        )
                        if jt == JT - 1:
                            # Phase D interleaved: acc `it` is final here, so
                            # out = num/Z + fc_b starts while accs it+1..7 are
                            # still accumulating their last strip.
                            rz = opool.tile([128, 1], f32, tag="rz")
                            nc.vector.reciprocal(
                                rz[:], out_ps[it][:, OUT_DIM : OUT_DIM + 1]
                            )
                            o = opool.tile([128, OUT_DIM], f32, tag="o")
                            nc.vector.tensor_scalar_mul(
                                o[:], out_ps[it][:, 0:OUT_DIM], rz[:]
                            )
                            o2 = opool.tile([128, OUT_DIM], f32, tag="o2")
                            nc.vector.tensor_add(o2[:], o[:], fcb_rep_sb[:])
                            nc.sync.dma_start(
                                out_d[it * 128 : (it + 1) * 128, :], o2[:]
                            )

    nc.compile()
    return nc


def _prep_inputs(adj, x, fc_w, fc_b, attn_w, attn_b):
    fc_w = np.asarray(fc_w, np.float32)
    fc_b = np.asarray(fc_b, np.float32)
    attn_w = np.asarray(attn_w, np.float32)
    x = np.asarray(x, np.float32)
    a_src = fc_w @ attn_w[:OUT_DIM]
    a_dst = fc_w @ attn_w[OUT_DIM:]
    b_src = float(fc_b @ attn_w[:OUT_DIM]) + float(attn_b)
    b_dst = float(fc_b @ attn_w[OUT_DIM:])

    xT = np.ascontiguousarray(x.T).astype(bf16)
    adjT = np.asarray(adj, np.float32).T  # [N (src j), N (dest i)]
    rhs_aug = np.concatenate(
        [fc_w, np.zeros((IN_DIM, 1), np.float32), a_dst[:, None]], axis=1
    ).astype(bf16)
    w_src_rep = np.tile(a_src[:, None], (1, 128)).astype(bf16)
    src_bias = np.full((128, 1), b_src, np.float32)

    # hsum = sum_j h_aug[j] over the de-biased h0 = x@fc_w (fc_b added in D);
    # dst column of h_aug includes b_dst, ones column sums to N.
    xsum = x.sum(0).astype(np.float64)
    hsum_h = xsum @ fc_w.astype(np.float64)  # [256]
    hsum_full = np.concatenate([hsum_h, [float(N)]]).astype(np.float32)  # [257]
    hsum_hi = hsum_full.astype(bf16)
    hsum_lo = (hsum_full - hsum_hi.astype(np.float32)).astype(bf16)
    # rows 0-63 = hi, rows 64-127 = lo; contracted against a (1/64) lhsT
    hsum = np.concatenate(
        [np.tile(hsum_hi[None, :], (64, 1)), np.tile(hsum_lo[None, :], (64, 1))]
    ).astype(bf16)  # [128, 257]
    fcb_rep = np.tile(fc_b[None, :], (128, 1)).astype(np.float32)

    gp_strips = sorted(_gp_strips())
    bf_strips = [j for j in range(JT) if j not in set(gp_strips)]
    in_maps = []
    for c in range(NCORES):
        sl = slice(c * R, (c + 1) * R)
        adjTc = adjT[:, sl]
        adjTb = np.concatenate(
            [adjTc[j * 128 : (j + 1) * 128] for j in bf_strips], axis=0
        ).astype(bf16)
        m = {
            "adjTb": np.ascontiguousarray(adjTb),
            "xT": xT,
            "xTi": np.ascontiguousarray(xT[:, sl]),
            "rhs_aug": rhs_aug,
            "w_src_rep": w_src_rep,
            "src_bias": src_bias,
            "hsum": hsum,
            "fcb_rep": fcb_rep,
        }
        if gp_strips:
            adjT8 = np.concatenate(
                [adjTc[j * 128 : (j + 1) * 128] for j in gp_strips], axis=0
            ).astype(f8)
            m["adjT8"] = np.ascontiguousarray(adjT8)
        in_maps.append(m)
    return in_maps


def kernel(adj, x, fc_w, fc_b, attn_w, attn_b, _trace=False, _tmpdir=None):
    from concourse import bass_utils

    if "nc" not in _cache:
        _cache["nc"] = _build()
    nc = _cache["nc"]
    in_maps = _prep_inputs(adj, x, fc_w, fc_b, attn_w, attn_b)
    res = bass_utils.run_bass_kernel_spmd(
        nc,
        in_maps,
        core_ids=list(range(NCORES)),
        trace=_trace,
        **({"tmpdir": _tmpdir} if _tmpdir else {}),
    )
    out = np.concatenate([res.results[c]["out"] for c in range(NCORES)], axis=0)
    if _trace:
        _cache["last_exec_time_ns"] = res.exec_time_ns
        _cache["last_profile_json"] = res.profile_json
    return out


# revision 40
# speedup vs baseline: 1.0714x; 1.0230x over previous
"""GAT layer (dense-adj variant) on 8 Trainium2 NeuronCores.

Row-parallel over destination nodes (R=1024 rows/core). Key structure vs the
old kernel: the score matrix E = exp(leaky_relu(src_i + dst_j)) * adj is
accumulated as G = E - 1 (G-decomposition), with the all-ones part folded in
exactly via a host-computed column sum of h:

  exp(leaky(z)) = max(exp(z), exp(0.01 z));  exp(z) = exp(src)*exp(dst)
  E - 1 = Prelu(exp(src_i)*exp(dst_j) - 1, alpha) * adj   (alpha~0.016 approx
          of the negative branch; end-to-end rel err ~1.8e-3)
  out_num[i,:] = hsum + sum_j G[j,i] h_aug[j,:]   (hsum = sum_j h_aug[j] from
          host x.sum(0) @ rhs, entering as two exact bf16 hi/lo K=1 matmuls
          that start each PSUM accumulation chain)
  out = out_num[:, :256]/Z + fc_b  (fc_b passes through softmax exactly since
          attention rows sum to 1); Z = out_num[:, 256].

Per j-strip cost: 1 ACT pass (Prelu, scale=exp(dst_j), bias=-1) + 1 mul by
adj (DVE or GPSIMD; GPSIMD-mul strips load adj as fp8 - GPSIMD rate is
dtype-blind, halving those strips' DMA). No per-element Exp/Prelu over the
full matrix, no softmax pass: ScalarE drops from ~105us to ~73us, DVE from
~110us to ~80us, adj DMA from 16.8MB to 13.3MB/core. PE is the wall
(~103us busy: C 57.5 + B 42 + LDW edges); B stays replicated because the
h all-gather alternatives measured too slow (CC AllGather ~20us/MB serial).

Phase B k-chains rotate over 4 full-bank-padded PSUM tiles (read-modify-
write on the same bank with rotation distance <=2 runs 40-60% slower), and
group g's elementwise is emitted while group g+1's B matmuls run. Pool
sizes (gstream=26, work=4) are load-bearing: larger pools increase engine
overlap and SBUF port contention (DVE tensor_tensor measured 620ns idle-PE
vs 1100-1700ns under load), measurably hurting wall time.

Measured: ~152-156us (run-to-run spread 149.5-158, +-3%), rel err 1.67e-3
(vs 179.6us / 3.58e-3 for the previous kernel). Rejected with hardware
measurements: fp8 adj/G on the DVE-mul path (1-byte operands drop DVE out
of 2x mode, ~1.7x slower in situ, outweighing halved DMA); fp8 DoubleRow
matmul for phase C (LDWEIGHTS-bound when the stationary operand changes
every matmul, 225ns per K=256 block = no gain); sharding phase B with
collective_compute AllGather (~20us per MB, serialized); emitting B-group-0
before phase A plus a 34-deep G pool (more concurrent SBUF traffic, net
loss). Run-to-run variance on these cores is +-3-5%: re-measure 3x before
trusting any single-digit-percent scheduling change.
"""

import numpy as np
import ml_dtypes

N = 8192
IN_DIM = 512
OUT_DIM = 256
NCORES = 8
R = N // NCORES  # 1024 rows per core
KT = IN_DIM // 128  # 4 k-tiles
JT = N // 128  # 64 j-strips
IT = R // 128  # 8 i-tiles per core
HS = OUT_DIM + 2  # h_sb slot width  (h | ones | dst)
HA = OUT_DIM + 1  # C-matmul rhs width (h | ones)
GC = 8  # strips per dst-extraction chunk

ALPHA = 0.016  # Prelu negative-branch slope approximating exp(0.01 z)-1
N_GP = 26  # strips whose adj-mul runs on GPSIMD (their adj comes in fp8)

bf16 = ml_dtypes.bfloat16
f8 = ml_dtypes.float8_e4m3fn

_cache = {}


def _gp_strips():
    return set(np.linspace(0, JT - 1, N_GP).astype(int).tolist()) if N_GP else set()


def _build():
    import concourse.tile as tile
    from concourse import bacc, mybir

    AF = mybir.ActivationFunctionType
    ALU = mybir.AluOpType
    f32 = mybir.dt.float32
    bft = mybir.dt.bfloat16
    f8t = mybir.dt.float8e4

    gp_strips = _gp_strips()

    nc = bacc.Bacc("TRN2", num_devices=NCORES, target_bir_lowering=False, debug=False)

    # adj strips: bf16 for DVE-mul strips (a 1-byte operand forces the DVE out
    # of 2x mode, ~1.7x slower in situ), fp8 for GPSIMD-mul strips (dtype-
    # blind engine, so the halved DMA is free there).
    n_gp = len(gp_strips)
    adjTb_d = nc.dram_tensor("adjTb", [(JT - n_gp) * 128, R], bft, kind="ExternalInput").ap()
    adjT8_d = (
        nc.dram_tensor("adjT8", [n_gp * 128, R], f8t, kind="ExternalInput").ap()
        if n_gp
        else None
    )
    xT_d = nc.dram_tensor("xT", [IN_DIM, N], bft, kind="ExternalInput").ap()
    xTi_d = nc.dram_tensor("xTi", [IN_DIM, R], bft, kind="ExternalInput").ap()
    # rhs_aug columns: [fc_w (256) | zeros (1) | a_dst (1)]
    rhs_aug_d = nc.dram_tensor("rhs_aug", [IN_DIM, HS], bft, kind="ExternalInput").ap()
    w_src_rep_d = nc.dram_tensor("w_src_rep", [IN_DIM, 128], bft, kind="ExternalInput").ap()
    src_bias_d = nc.dram_tensor("src_bias", [128, 1], f32, kind="ExternalInput").ap()
    # hsum replicated (hi in rows 0-63, lo in rows 64-127): one K=128 matmul
    # against a constant (1/64) lhsT adds hi+lo exactly into each acc chain.
    hsum_d = nc.dram_tensor("hsum", [128, HA], bft, kind="ExternalInput").ap()
    fcb_rep_d = nc.dram_tensor("fcb_rep", [128, OUT_DIM], f32, kind="ExternalInput").ap()
    out_d = nc.dram_tensor("out", [R, OUT_DIM], f32, kind="ExternalOutput").ap()

    with tile.TileContext(nc) as tc:
        with (
            tc.tile_pool(name="const", bufs=1) as cpool,
            tc.tile_pool(name="hpool", bufs=1) as hpool,
            tc.tile_pool(name="xstream", bufs=3) as xpool,
            tc.tile_pool(name="abstream", bufs=10) as abpool,
            tc.tile_pool(name="a8stream", bufs=6) as a8pool,
            tc.tile_pool(name="gstream", bufs=26) as gpool,
            tc.tile_pool(name="work", bufs=4) as wpool,
            tc.tile_pool(name="opool", bufs=3) as opool,
        ):
            # ---- constants (xTi/w_src first: phase A gates the first Prelu) ----
            src_bias_sb = cpool.tile([128, 1], f32)
            nc.sync.dma_start(src_bias_sb[:], src_bias_d)
            w_src_sb = cpool.tile([128, KT * 128], bft)
            nc.sync.dma_start(
                w_src_sb[:].rearrange("p (k n) -> p k n", k=KT),
                w_src_rep_d.rearrange("(k p) n -> p k n", p=128),
            )
            xTi_sb = cpool.tile([128, KT * R], bft)
            for half in range(2):
                nc.sync.dma_start(
                    xTi_sb[:].rearrange("p (k n) -> p k n", k=KT)[
                        :, :, half * 512 : (half + 1) * 512
                    ],
                    xTi_d.rearrange("(k p) n -> p k n", p=128)[
                        :, :, half * 512 : (half + 1) * 512
                    ],
                )
            rhs_aug_sb = cpool.tile([128, KT * HS], bft)
            nc.sync.dma_start(
                rhs_aug_sb[:].rearrange("p (k n) -> p k n", k=KT),
                rhs_aug_d.rearrange("(k p) n -> p k n", p=128),
            )
            hsum_sb = cpool.tile([128, HA], bft)
            nc.sync.dma_start(hsum_sb[:], hsum_d)
            fcb_rep_sb = cpool.tile([128, OUT_DIM], f32)
            nc.sync.dma_start(fcb_rep_sb[:], fcb_rep_d)
            inv64_sb = cpool.tile([128, 128], bft)
            nc.vector.memset(inv64_sb[:], 1.0 / 64.0)
            neg1_sb = cpool.tile([128, 1], f32)
            nc.vector.memset(neg1_sb[:], -1.0)

            src_rep = cpool.tile([128, R], bft)
            esrc_rep = cpool.tile([128, R], bft)
            h_sb = hpool.tile([128, JT * HS], bft)
            dst_sb = cpool.tile([128, JT], f32)
            edst_sb = cpool.tile([128, JT], f32)
            g_strips = [None] * JT

            ps_ab_cm = tc.tile_pool(name="ps_ab", bufs=8, space="PSUM")
            ps_ab = ps_ab_cm.__enter__()

            # ---- Phase B (replicated h) + per-strip elementwise ----
            bi = 0  # running index into adjTb
            g8i = 0  # running index into adjT8
            adj_tiles = [None] * JT

            def c_adj_dma(jt):
                nonlocal bi, g8i
                if jt in gp_strips:
                    adjt = a8pool.tile([128, R], f8t, name="adj8")
                    nc.sync.dma_start(adjt[:], adjT8_d[g8i * 128 : (g8i + 1) * 128, :])
                    g8i += 1
                else:
                    adjt = abpool.tile([128, R], bft, name="adjb")
                    nc.sync.dma_start(adjt[:], adjTb_d[bi * 128 : (bi + 1) * 128, :])
                    bi += 1
                adj_tiles[jt] = adjt

            def c_elementwise(jt):
                # G[j,i] strip: Prelu(exp(src)*exp(dst_j) - 1, alpha) * adj
                pre = wpool.tile([128, R], bft, name="pre", tag="pre")
                nc.scalar.activation(
                    pre[:], esrc_rep[:], AF.Prelu,
                    bias=neg1_sb[:], scale=edst_sb[:, jt : jt + 1], alpha=ALPHA,
                )
                g = gpool.tile([128, R], bft, name="g")
                if jt in gp_strips:
                    nc.gpsimd.tensor_mul(g[:], pre[:], adj_tiles[jt][:])
                else:
                    nc.vector.tensor_mul(g[:], pre[:], adj_tiles[jt][:])
                g_strips[jt] = g

            def c_group_post(g0):
                # ones column + dst extraction + exp for group g0's slots
                nc.vector.memset(
                    h_sb[:, g0 * GC * HS : (g0 + 1) * GC * HS].rearrange(
                        "p (j s) -> p j s", s=HS
                    )[:, :, OUT_DIM : OUT_DIM + 1],
                    1.0,
                )
                nc.vector.tensor_copy(
                    dst_sb[:, g0 * GC : (g0 + 1) * GC],
                    h_sb[:, g0 * GC * HS : (g0 + 1) * GC * HS].rearrange(
                        "p (j s) -> p j s", s=HS
                    )[:, :, HS - 1 : HS],
                )
                nc.scalar.activation(
                    edst_sb[:, g0 * GC : (g0 + 1) * GC],
                    dst_sb[:, g0 * GC : (g0 + 1) * GC],
                    AF.Exp,
                )

            # 8 strips (one group) per iteration; ps tiles padded to [128,512]
            # f32 = one full 2KB bank each, so the k-chains rotate over all 8
            # PSUM banks.
            def b_group(jq):
                xTj = xpool.tile([128, KT * GC * 128], bft)
                nc.sync.dma_start(
                    xTj[:].rearrange("p (k n) -> p k n", k=KT),
                    xT_d[:, jq * GC * 128 : (jq + 1) * GC * 128].rearrange(
                        "(k p) n -> p k n", p=128
                    ),
                )
                ps8 = [ps_ab.tile([128, 512], f32, name=f"ps_b{u}", tag="ps")
                       for u in range(GC)]
                for kt in range(KT):
                    for u in range(GC):
                        nc.tensor.matmul(
                            ps8[u][:, 0:HS],
                            xTj[:, (kt * GC + u) * 128 : (kt * GC + u + 1) * 128],
                            rhs_aug_sb[:, kt * HS : (kt + 1) * HS],
                            start=(kt == 0),
                            stop=(kt == KT - 1),
                        )
                # slot: [h (256) | 1.0 | dst]; rhs_aug col 256 is 0 so the copy
                # writes 0 there; a strided memset per group sets the ones.
                for u in range(GC):
                    jt_ = jq * GC + u
                    nc.vector.tensor_copy(
                        h_sb[:, jt_ * HS : (jt_ + 1) * HS], ps8[u][:, 0:HS]
                    )
                    c_adj_dma(jt_)
                c_group_post(jq)

            # ---- Phase A: src_rep[p, f] = src[i0+f] for all p; esrc = exp ----
            ps_a = [ps_ab.tile([128, 512], f32, name=f"ps_a{ch}", tag="ps")
                    for ch in range(R // 512)]
            for kt in range(KT):
                for ch in range(R // 512):
                    nc.tensor.matmul(
                        ps_a[ch][:],
                        w_src_sb[:, kt * 128 : (kt + 1) * 128],
                        xTi_sb[:, kt * R + ch * 512 : kt * R + (ch + 1) * 512],
                        start=(kt == 0),
                        stop=(kt == KT - 1),
                    )
            for ch in range(R // 512):
                nc.scalar.activation(
                    src_rep[:, ch * 512 : (ch + 1) * 512], ps_a[ch][:], AF.Identity,
                    bias=src_bias_sb[:],
                )
            nc.scalar.activation(esrc_rep[:], src_rep[:], AF.Exp)

            for jq in range(JT // GC):
                b_group(jq)
                for s_jt in range(jq * GC, (jq + 1) * GC):
                    c_elementwise(s_jt)

            # ---- Phase C: 8 PSUM bank accumulators, hsum-start + 64 strips ----
            ps_ab_cm.__exit__(None, None, None)
            out_ps = {}
            with tc.tile_pool(name="ps_acc", bufs=1, space="PSUM") as ps_acc:
                for it in range(IT):
                    out_ps[it] = ps_acc.tile(
                        [128, HA], f32, name=f"acc{it}", tag=f"acc{it}"
                    )
                for it in range(IT):
                    # acc := (1/64) * ones^T @ hsum_rep = hsum_hi + hsum_lo
                    nc.tensor.matmul(
                        out_ps[it][:], inv64_sb[:], hsum_sb[:],
                        start=True, stop=False,
                    )
                for jt in range(JT):
                    g = g_strips[jt]
                    hj = h_sb[:, jt * HS : jt * HS + HA]
                    for it in range(IT):
                        nc.tensor.matmul(
                            out_ps[it][:],
                            g[:, it * 128 : (it + 1) * 128],
                            hj,
                            start=False,
                            stop=(jt == JT - 1),
                        )
                        if jt == JT - 1:
                            # Phase D interleaved: acc `it` is final here, so
                            # out = num/Z + fc_b starts while accs it+1..7 are
                            # still accumulating their last strip.
                            rz = opool.tile([128, 1], f32, tag="rz")
                            nc.vector.reciprocal(
                                rz[:], out_ps[it][:, OUT_DIM : OUT_DIM + 1]
                            )
                            o = opool.tile([128, OUT_DIM], f32, tag="o")
                            nc.vector.tensor_scalar_mul(
                                o[:], out_ps[it][:, 0:OUT_DIM], rz[:]
                            )
                            o2 = opool.tile([128, OUT_DIM], f32, tag="o2")
                            nc.vector.tensor_add(o2[:], o[:], fcb_rep_sb[:])
                            nc.sync.dma_start(
                                out_d[it * 128 : (it + 1) * 128, :], o2[:]
                            )

    nc.compile()
    return nc


def _prep_inputs(adj, x, fc_w, fc_b, attn_w, attn_b):
    fc_w = np.asarray(fc_w, np.float32)
    fc_b = np.asarray(fc_b, np.float32)
    attn_w = np.asarray(attn_w, np.float32)
    x = np.asarray(x, np.float32)
    a_src = fc_w @ attn_w[:OUT_DIM]
    a_dst = fc_w @ attn_w[OUT_DIM:]
    b_src = float(fc_b @ attn_w[:OUT_DIM]) + float(attn_b)
    b_dst = float(fc_b @ attn_w[OUT_DIM:])

    xT = np.ascontiguousarray(x.T).astype(bf16)
    adjT = np.asarray(adj, np.float32).T  # [N (src j), N (dest i)]
    rhs_aug = np.concatenate(
        [fc_w, np.zeros((IN_DIM, 1), np.float32), a_dst[:, None]], axis=1
    ).astype(bf16)
    w_src_rep = np.tile(a_src[:, None], (1, 128)).astype(bf16)
    src_bias = np.full((128, 1), b_src, np.float32)

    # hsum = sum_j h_aug[j] over the de-biased h0 = x@fc_w (fc_b added in D);
    # dst column of h_aug includes b_dst, ones column sums to N.
    xsum = x.sum(0).astype(np.float64)
    hsum_h = xsum @ fc_w.astype(np.float64)  # [256]
    hsum_full = np.concatenate([hsum_h, [float(N)]]).astype(np.float32)  # [257]
    hsum_hi = hsum_full.astype(bf16)
    hsum_lo = (hsum_full - hsum_hi.astype(np.float32)).astype(bf16)
    # rows 0-63 = hi, rows 64-127 = lo; contracted against a (1/64) lhsT
    hsum = np.concatenate(
        [np.tile(hsum_hi[None, :], (64, 1)), np.tile(hsum_lo[None, :], (64, 1))]
    ).astype(bf16)  # [128, 257]
    fcb_rep = np.tile(fc_b[None, :], (128, 1)).astype(np.float32)

    gp_strips = sorted(_gp_strips())
    bf_strips = [j for j in range(JT) if j not in set(gp_strips)]
    in_maps = []
    for c in range(NCORES):
        sl = slice(c * R, (c + 1) * R)
        adjTc = adjT[:, sl]
        adjTb = np.concatenate(
            [adjTc[j * 128 : (j + 1) * 128] for j in bf_strips], axis=0
        ).astype(bf16)
        m = {
            "adjTb": np.ascontiguousarray(adjTb),
            "xT": xT,
            "xTi": np.ascontiguousarray(xT[:, sl]),
            "rhs_aug": rhs_aug,
            "w_src_rep": w_src_rep,
            "src_bias": src_bias,
            "hsum": hsum,
            "fcb_rep": fcb_rep,
        }
        if gp_strips:
            adjT8 = np.concatenate(
                [adjTc[j * 128 : (j + 1) * 128] for j in gp_strips], axis=0
            ).astype(f8)
            m["adjT8"] = np.ascontiguousarray(adjT8)
        in_maps.append(m)
    return in_maps


def kernel(adj, x, fc_w, fc_b, attn_w, attn_b, _trace=False, _tmpdir=None):
    from concourse import bass_utils

    if "nc" not in _cache:
        _cache["nc"] = _build()
    nc = _cache["nc"]
    in_maps = _prep_inputs(adj, x, fc_w, fc_b, attn_w, attn_b)
    res = bass_utils.run_bass_kernel_spmd(
        nc,
        in_maps,
        core_ids=list(range(NCORES)),
        trace=_trace,
        **({"tmpdir": _tmpdir} if _tmpdir else {}),
    )
    out = np.concatenate([res.results[c]["out"] for c in range(NCORES)], axis=0)
    if _trace:
        _cache["last_exec_time_ns"] = res.exec_time_ns
        _cache["last_profile_json"] = res.profile_json
    return out
